# revision 17
# baseline (speedup 1.0000x reference)
"""Deformable conv net kernel for 8 TRN2 NeuronCores (data-parallel over batch).

v3: gather-before-matmul. Per core (one batch sample):
  1. offsets via transposed 3x3 conv (out free dim = 18)      (PE)
  2. bilinear fields: corner indices + weights, pixel-major   (DVE)
  3. SWDGE pair-gather of x channel rows from host-prepared
     xT [HW, C] in DRAM (2 descs of 512B per pixel/tap)       (Pool+DMA)
  4. S^T[c,p] += gt^T @ diag(w): scale+accumulate+transpose
     in one matmul per (chunk,k,corner)                       (PE, diag on DVE/ACT)
  5. out^T[o,p] = sum_k wmain_k^T @ S_k^T + bias              (PE)
  6. host reassembles [8, 128, 64, 64] from out^T [O, HW].
"""
import os, sys

for _p in ("/opt/trn_rl_repo", "/root/.axon_site/_ro/trn_rl_repo"):
    if os.path.isdir(_p) and _p not in sys.path:
        sys.path.insert(0, _p)

import numpy as np
import ml_dtypes

import concourse.bass as bass
import concourse.mybir as mybir
from concourse import bacc, library_config
from concourse.tile import TileContext

BF16 = mybir.dt.bfloat16
F32 = mybir.dt.float32
I16 = mybir.dt.int16

B, C, H, W = 8, 128, 64, 64
O = 128
K = 3
K2 = 9
HW = H * W                 # 4096
NCH = HW // 128            # 32 pixel chunks of 128
NH = 2                     # halves of the pixel space for the gather phase
CPH = NCH // NH            # 16 chunks per half
GP = 66                    # guarded row pitch of xg
XG = (H + 2) * GP          # guarded image cols
FDIM = NCH * K2            # 288
MAGIC = float(3 * 2 ** 22)  # 1.5*2^23: keeps s+M in the ulp=1 binade

_MAX_WAITS = 1             # this walrus build rejects >1 sem wait per inst


def _split_excess_waits(nc):
    for f in nc.m.functions:
        for bb in f.blocks:
            new_insts = []
            for inst in bb.instructions:
                si = inst.sync_info
                if si is not None and si.on_wait and len(si.on_wait) > _MAX_WAITS:
                    waits = list(si.on_wait)
                    keep = waits[-_MAX_WAITS:]
                    spill = waits[:-_MAX_WAITS]
                    for j in range(0, len(spill), _MAX_WAITS):
                        chunk = spill[j:j + _MAX_WAITS]
                        nop = mybir.InstNoOp(
                            name=f"{inst.name}-wsp{j}",
                            engine=inst.engine,
                            ins=[], outs=[],
                            sync_info=mybir.SyncInfo(on_wait=chunk, on_update=[]),
                        )
                        nc.register_instruction(nop, overwrite=True)
                        new_insts.append(nop)
                    inst.sync_info = mybir.SyncInfo(
                        on_wait=keep, on_update=list(si.on_update or []))
                new_insts.append(inst)
            bb.instructions[:] = new_insts


def build_nc(act_diag_mod=5, gtbufs=3, dgbufs=10, ngsplit=2):
    nc = bacc.Bacc()
    xg_in = nc.dram_tensor("xg", [C, XG], BF16, kind="ExternalInput")
    xt_in = nc.dram_tensor("xt", [HW, C], BF16, kind="ExternalInput")
    offw_in = nc.dram_tensor("offw", [C, K2 * 18], BF16, kind="ExternalInput")
    wmain_in = nc.dram_tensor("wmain", [C, K2 * O], BF16, kind="ExternalInput")
    biaso_in = nc.dram_tensor("biaso", [128, 1], F32, kind="ExternalInput")
    ybase_in = nc.dram_tensor("ybase", [128, FDIM], F32, kind="ExternalInput")
    xbase_in = nc.dram_tensor("xbase", [128, FDIM], F32, kind="ExternalInput")
    idb_in = nc.dram_tensor("identb", [128, 128], BF16, kind="ExternalInput")
    out_dram = nc.dram_tensor("out", [O, HW], F32, kind="ExternalOutput")

    with TileContext(nc) as tc:
        with tc.tile_pool(name="cst", bufs=1) as cst, \
             tc.tile_pool(name="fld", bufs=1) as fld, \
             tc.tile_pool(name="gth", bufs=gtbufs) as gth, \
             tc.tile_pool(name="dgp", bufs=dgbufs) as dgp, \
             tc.tile_pool(name="stb", bufs=1) as stb, \
             tc.tile_pool(name="otb", bufs=2) as otb:

            nc.gpsimd.load_library(library_config.mlp)

            # Tiny SWDGE op up front: bass barriers POOL's first dynamic DMA
            # against ALL outstanding HWDGE lanes; firing it now (nothing in
            # flight) keeps that barrier off the gather critical path.
            warm = cst.tile([16, 16], BF16, name="warm")
            nc.gpsimd.dma_start(warm[:, :], xg_in[0:16, 0:16])

            # ---- constant / input loads ----
            offw_sb = cst.tile([C, K2 * 18], BF16, name="offw_sb")
            nc.sync.dma_start(offw_sb[:, :], offw_in[:, :])
            wmain_sb = cst.tile([C, K2 * O], BF16, name="wmain_sb")
            nc.sync.dma_start(wmain_sb[:, :], wmain_in[:, :])
            biaso_sb = cst.tile([128, 1], F32, name="biaso_sb")
            nc.sync.dma_start(biaso_sb[:, :], biaso_in[:, :])
            ybase_sb = cst.tile([128, FDIM], F32, name="ybase_sb")
            nc.sync.dma_start(ybase_sb[:, :], ybase_in[:, :])
            xbase_sb = cst.tile([128, FDIM], F32, name="xbase_sb")
            nc.sync.dma_start(xbase_sb[:, :], xbase_in[:, :])
            identb = cst.tile([128, 128], BF16, name="identb")
            nc.sync.dma_start(identb[:, :], idb_in[:, :])
            xg_sb = cst.tile([C, XG], BF16, name="xg_sb")
            nc.sync.dma_start(xg_sb[:, :], xg_in[:, :])

            # ---- offset conv, transposed: offT[p, c*18 + j] ----
            # lhsT = guarded x pixels for chunk c shifted by tap (ky,kx);
            # rhs = offw tap slice [C, 18]; out free dim = 18.
            offT = fld.tile([128, NCH * 18], F32, name="offT")
            xg3 = xg_sb[:, :].rearrange("c (r w) -> c r w", w=GP)
            pso_cm = tc.tile_pool(name="pso", bufs=2, space="PSUM")
            pso = pso_cm.__enter__()
            for cg in range(8):
                ps = pso.tile([128, 8 * 18], F32, name=f"offps{cg}", tag="offps")
                ps_r = ps[:, :].rearrange("p (c4 two j) -> p two c4 j", two=2, j=18)
                for c4 in range(4):
                    c = cg * 4 + c4
                    for r in range(2):
                        for k in range(K2):
                            ky, kx = k // 3, k % 3
                            lhs = xg3[:, 2 * c + r + ky, kx: kx + 64]
                            nc.tensor.matmul(
                                ps[r * 64:(r + 1) * 64,
                                   (c4 * 2 + r) * 18:(c4 * 2 + r + 1) * 18],
                                lhs,
                                offw_sb[:, k * 18:(k + 1) * 18],
                                start=(k == 0), stop=(k == K2 - 1))
                offT_r = offT[:, :].rearrange("p (c j) -> p c j", j=18)
                for r in range(2):
                    nc.vector.tensor_copy(
                        offT_r[r * 64:(r + 1) * 64, cg * 4:(cg + 1) * 4],
                        ps_r[r * 64:(r + 1) * 64, r])
            pso_cm.__exit__(None, None, None)

            # ---- bilinear fields (fp32, [128, (c,k)=288]) ----
            offT4 = offT[:, :].rearrange("p (c k two) -> p two c k", two=2, k=K2)
            yb3 = ybase_sb[:, :].rearrange("p (c k) -> p c k", k=K2)
            xb3 = xbase_sb[:, :].rearrange("p (c k) -> p c k", k=K2)

            def f3(name):
                t = fld.tile([128, FDIM], F32, name=name, tag=name)
                return t, t[:, :].rearrange("p (c k) -> p c k", k=K2)

            VA = mybir.AluOpType
            axes = {}
            axes_i0 = {}
            for ax in ("y", "x"):
                s, s3 = f3(f"s_{ax}")
                base3 = yb3 if ax == "y" else xb3
                nc.vector.tensor_tensor(s3, offT4[:, 0 if ax == "y" else 1], base3, VA.add)
                r, r3 = f3(f"r_{ax}")
                nc.vector.tensor_scalar_add(r[:, :], s[:, :], MAGIC)
                nc.vector.tensor_scalar_add(r[:, :], r[:, :], -MAGIC)
                g, g3 = f3(f"g_{ax}")
                nc.vector.tensor_tensor(g[:, :], r[:, :], s[:, :], VA.is_gt)
                i0, _ = f3(f"i0_{ax}")
                nc.vector.tensor_tensor(i0[:, :], r[:, :], g[:, :], VA.subtract)
                fr, _ = f3(f"fr_{ax}")
                nc.vector.tensor_tensor(fr[:, :], s[:, :], i0[:, :], VA.subtract)
                i1, _ = f3(f"i1_{ax}")
                nc.vector.tensor_scalar_add(i1[:, :], i0[:, :], 1.0)
                w_m = []
                for (ii, frac_is_w) in ((i0, False), (i1, True)):
                    v, _ = f3(f"v_{ax}_{frac_is_w}")
                    nc.vector.tensor_scalar(v[:, :], ii[:, :], 0.0, None, VA.is_ge)
                    t2, _ = f3(f"t2_{ax}_{frac_is_w}")
                    nc.vector.tensor_scalar(t2[:, :], ii[:, :], float(H - 1), None, VA.is_le)
                    nc.vector.tensor_tensor(v[:, :], v[:, :], t2[:, :], VA.mult)
                    wm, _ = f3(f"wm_{ax}_{frac_is_w}")
                    if frac_is_w:
                        nc.vector.tensor_tensor(wm[:, :], fr[:, :], v[:, :], VA.mult)
                    else:
                        nc.vector.tensor_scalar(wm[:, :], fr[:, :], -1.0, 1.0,
                                                VA.mult, VA.add)
                        nc.vector.tensor_tensor(wm[:, :], wm[:, :], v[:, :], VA.mult)
                    w_m.append(wm)
                cl = []
                for ii in (i0, i1):
                    cc, _ = f3(f"c_{ax}_{ii is i1}")
                    nc.vector.tensor_scalar(cc[:, :], ii[:, :], 0.0, float(H - 1),
                                            VA.max, VA.min)
                    cl.append(cc)
                axes[ax] = (w_m, cl)
                axes_i0[ax] = i0

            (wy, cy), (wx, _cxunused) = axes["y"], axes["x"]
            # pair-fetch base bx = clip(ix0, 0, 62); weights for pair slots
            ix0f = axes_i0["x"]
            bx, _ = f3("bx")
            nc.vector.tensor_scalar(bx[:, :], ix0f[:, :], 0.0, float(W - 2),
                                    VA.max, VA.min)
            dif, _ = f3("dif")
            nc.vector.tensor_tensor(dif[:, :], bx[:, :], ix0f[:, :], VA.subtract)
            eqA, _ = f3("eqA")
            nc.vector.tensor_scalar(eqA[:, :], dif[:, :], 0.0, None, VA.is_equal)
            eqB, _ = f3("eqB")
            nc.vector.tensor_scalar(eqB[:, :], dif[:, :], 1.0, None, VA.is_equal)
            eqC, _ = f3("eqC")
            nc.vector.tensor_scalar(eqC[:, :], dif[:, :], -1.0, None, VA.is_equal)
            WL, _ = f3("WL")
            WR, _ = f3("WR")
            t1, _ = f3("t1")
            nc.vector.tensor_tensor(WL[:, :], wx[0][:, :], eqA[:, :], VA.mult)
            nc.vector.tensor_tensor(t1[:, :], wx[1][:, :], eqB[:, :], VA.mult)
            nc.vector.tensor_tensor(WL[:, :], WL[:, :], t1[:, :], VA.add)
            nc.vector.tensor_tensor(WR[:, :], wx[1][:, :], eqA[:, :], VA.mult)
            nc.vector.tensor_tensor(t1[:, :], wx[0][:, :], eqC[:, :], VA.mult)
            nc.vector.tensor_tensor(WR[:, :], WR[:, :], t1[:, :], VA.add)
            # weights per (a, side): wcor2[a*2+side]
            wcor2 = []
            for a in range(2):
                for sd, Wside in ((0, WL), (1, WR)):
                    wc, _ = f3(f"wc{a}{sd}")
                    nc.vector.tensor_tensor(wc[:, :], wy[a][:, :], Wside[:, :], VA.mult)
                    wcor2.append(wc)
            # pair row indices idx = cy*64 + bx
            cys = []
            for a in range(2):
                cs, _ = f3(f"cys{a}")
                nc.vector.tensor_scalar_mul(cs[:, :], cy[a][:, :], float(W))
                cys.append(cs)
            # fidx col = ((k*2+a)*NH + h)*CPH + j  (chunk c = h*CPH + j)
            fidx = fld.tile([128, 2 * FDIM], F32, name="fidx")
            fidx_r = fidx[:, :].rearrange("p (k a h j) -> p a h j k",
                                          k=K2, a=2, h=NH, j=CPH)
            for a in range(2):
                nc.vector.tensor_tensor(fidx_r[:, a],
                                        cys[a][:, :].rearrange(
                                            "p (h j k) -> p h j k",
                                            h=NH, j=CPH, k=K2),
                                        bx[:, :].rearrange(
                                            "p (h j k) -> p h j k",
                                            h=NH, j=CPH, k=K2), VA.add)
            fidxi = fld.tile([128, 2 * FDIM], I16, name="fidxi")
            nc.vector.tensor_copy(fidxi[:, :], fidx[:, :])

            # ---- fold indices into SWDGE wrapped layout ----
            # idxw col = kahj*8 + f; value stream for (k,a,h): i = j*128 + p
            # -> wrapped (i%16 = p%16, i//16 = j*8 + p//16)
            idxw = fld.tile([128, K2 * 2 * NH * CPH * 8], I16, name="idxw")
            dst_r = idxw[:, :].rearrange("p (kahj f) -> p f kahj", f=8)
            # ACT HWDGE ring keeps these off the SP FIFO.
            for f in range(8):
                nc.scalar.dma_start(dst_r[0:16, f],
                                    fidxi[16 * f:16 * (f + 1), :])
            for f in range(1, 8):
                nc.scalar.dma_start(idxw[16 * f:16 * (f + 1), :], idxw[0:16, :])

            # ---- gather + diag-matmul accumulate + GEMM ----
            xsrc = xt_in[:, :]
            xpairs = bass.AP(tensor=xsrc.tensor, offset=xsrc.offset,
                             ap=[[C, HW - 1], [1, 2 * C]])
            psp_cm = tc.tile_pool(name="ps", bufs=2, space="PSUM")
            psp = psp_cm.__enter__()
            pso2_cm = tc.tile_pool(name="pso2", bufs=1, space="PSUM")
            pso2 = pso2_cm.__enter__()
            ndiag = 0
            for h in range(NH):
                st_sb = stb.tile([128, K2 * CPH * 128], BF16,
                                 name=f"st{h}", tag="st")
                for k in range(K2):
                    gts = []
                    for a in range(2):
                        gt = gth.tile([128, CPH, 2 * C], BF16,
                                      name=f"g{h}_{k}_{a}", tag="gath")
                        base = ((k * 2 + a) * NH + h) * CPH * 8
                        ni = CPH * 128 // ngsplit
                        for g2 in range(ngsplit):
                            cpg = CPH // ngsplit
                            nc.gpsimd.dma_gather(
                                gt[:, g2 * cpg:(g2 + 1) * cpg, :], xpairs,
                                idxw[:, base + g2 * cpg * 8:
                                     base + (g2 + 1) * cpg * 8],
                                ni, ni, 2 * C, elem_step=C)
                        gts.append(gt)
                    for q in range(2):
                        st_ps = psp.tile([128, 8 * 128], F32,
                                         name=f"sp{h}_{k}_{q}", tag="stps")
                        for j8 in range(8):
                            j = q * 8 + j8
                            c = h * CPH + j
                            for a in range(2):
                                for sd in range(2):
                                    wcol = wcor2[a * 2 + sd][:, c * K2 + k:
                                                             c * K2 + k + 1]
                                    dg = dgp.tile([128, 128], BF16,
                                                  name=f"d{h}_{k}_{q}_{a}_{sd}_{j8}",
                                                  tag="diag")
                                    if ndiag % act_diag_mod == 0:
                                        nc.scalar.activation(
                                            dg[:, :], identb[:, :],
                                            mybir.ActivationFunctionType.Copy,
                                            scale=wcol)
                                    else:
                                        nc.vector.tensor_scalar_mul(
                                            dg[:, :], identb[:, :], wcol)
                                    ndiag += 1
                                    nc.tensor.matmul(
                                        st_ps[:, j8 * 128:(j8 + 1) * 128],
                                        gts[a][:, j, sd * C:(sd + 1) * C],
                                        dg[:, :],
                                        start=(a == 0 and sd == 0),
                                        stop=(a == 1 and sd == 1))
                        nc.scalar.copy(
                            st_sb[:, (k * 2 + q) * 1024:(k * 2 + q + 1) * 1024],
                            st_ps[:, :])
                # GEMM: out^T[o, p] = sum_k wmain_k^T @ S_k^T, + bias
                ot_ps = pso2.tile([128, CPH * 128], F32, name=f"ot{h}", tag="otps")
                for j in range(CPH):
                    q, j8 = j // 8, j % 8
                    for k in range(K2):
                        nc.tensor.matmul(
                            ot_ps[:, j * 128:(j + 1) * 128],
                            wmain_sb[:, k * O:(k + 1) * O],
                            st_sb[:, (k * 2 + q) * 1024 + j8 * 128:
                                  (k * 2 + q) * 1024 + (j8 + 1) * 128],
                            start=(k == 0), stop=(k == K2 - 1))
                ot_sb = otb.tile([128, CPH * 128], F32, name=f"ots{h}", tag="ots")
                nc.vector.tensor_scalar_add(ot_sb[:, :], ot_ps[:, :],
                                            biaso_sb[:, 0:1])
                nc.sync.dma_start(out_dram[:, h * CPH * 128:(h + 1) * CPH * 128],
                                  ot_sb[:, :])
            pso2_cm.__exit__(None, None, None)
            psp_cm.__exit__(None, None, None)

    nc.compile()
    _split_excess_waits(nc)
    return nc


_NC_CACHE = None


def _get_nc():
    global _NC_CACHE
    if _NC_CACHE is None:
        _NC_CACHE = build_nc()
    return _NC_CACHE


def _host_inputs(x, offset_w, offset_b, weight, bias):
    bf = ml_dtypes.bfloat16
    offw = np.ascontiguousarray(
        offset_w.reshape(18, C, K2).transpose(1, 2, 0).reshape(C, K2 * 18)).astype(bf)
    wmain = np.ascontiguousarray(
        weight.reshape(O, C, K2).transpose(1, 2, 0).reshape(C, K2 * O)).astype(bf)
    biaso = bias.reshape(128, 1).astype(np.float32)
    pi = np.arange(128)
    cc = np.arange(NCH)
    kk = np.arange(K2)
    pix = cc[None, :, None] * 128 + pi[:, None, None]          # [128, 32, 1]
    ob = offset_b.reshape(K2, 2).astype(np.float32)
    ybase = (pix // W - 1 + (kk // 3)[None, None, :]
             + ob[None, None, :, 0]).reshape(128, FDIM).astype(np.float32)
    xbase = (pix % W - 1 + (kk % 3)[None, None, :]
             + ob[None, None, :, 1]).reshape(128, FDIM).astype(np.float32)
    identb = np.eye(128, dtype=bf)
    shared = dict(offw=offw, wmain=wmain, biaso=biaso,
                  ybase=ybase, xbase=xbase, identb=identb)
    maps = []
    for b in range(B):
        m = dict(shared)
        xb = x[b].reshape(C, H, W)
        P = np.zeros((C, H + 2, GP), np.float32)
        P[:, 1:H + 1, 1:W + 1] = xb
        m["xg"] = P.reshape(C, XG).astype(bf)
        m["xt"] = np.ascontiguousarray(x[b].reshape(C, HW).T).astype(bf)
        maps.append(m)
    return maps


def kernel(x, offset_w, offset_b, weight, bias):
    from concourse.bass_utils import run_bass_kernel_spmd
    nc = _get_nc()
    in_maps = _host_inputs(np.asarray(x, np.float32), np.asarray(offset_w, np.float32),
                           np.asarray(offset_b, np.float32),
                           np.asarray(weight, np.float32), np.asarray(bias, np.float32))
    res = run_bass_kernel_spmd(nc, in_maps, core_ids=list(range(B)))
    out = np.stack([np.asarray(res.results[b]["out"], np.float32).reshape(O, H, W)
                    for b in range(B)])
    return out


# revision 23
# speedup vs baseline: 1.5022x; 1.5022x over previous
"""Deformable conv net kernel for 8 TRN2 NeuronCores (data-parallel over batch).

v3: gather-before-matmul. Per core (one batch sample):
  1. offsets via transposed 3x3 conv (out free dim = 18)      (PE)
  2. bilinear fields: corner indices + weights, pixel-major   (DVE)
  3. SWDGE pair-gather of x channel rows from host-prepared
     xT [HW, C] in DRAM (2 descs of 512B per pixel/tap)       (Pool+DMA)
  4. S^T[c,p] += gt^T @ diag(w): scale+accumulate+transpose
     in one matmul per (chunk,k,corner)                       (PE, diag on DVE/ACT)
  5. out^T[o,p] = sum_k wmain_k^T @ S_k^T + bias              (PE)
  6. host reassembles [8, 128, 64, 64] from out^T [O, HW].
"""
import os, sys

for _p in ("/opt/trn_rl_repo", "/root/.axon_site/_ro/trn_rl_repo"):
    if os.path.isdir(_p) and _p not in sys.path:
        sys.path.insert(0, _p)

import numpy as np
import ml_dtypes

import concourse.bass as bass
import concourse.mybir as mybir
from concourse import bacc, library_config
from concourse.tile import TileContext

BF16 = mybir.dt.bfloat16
F32 = mybir.dt.float32
I16 = mybir.dt.int16

B, C, H, W = 8, 128, 64, 64
O = 128
K = 3
K2 = 9
HW = H * W                 # 4096
NCH = HW // 128            # 32 pixel chunks of 128
NH = 2                     # halves of the pixel space for the gather phase
CPH = NCH // NH            # 16 chunks per half
GP = 66                    # guarded row pitch of xg
XG = (H + 2) * GP          # guarded image cols
FDIM = NCH * K2            # 288
MAGIC = float(3 * 2 ** 22)  # 1.5*2^23: keeps s+M in the ulp=1 binade

_MAX_WAITS = 1             # this walrus build rejects >1 sem wait per inst


def _split_excess_waits(nc):
    for f in nc.m.functions:
        for bb in f.blocks:
            new_insts = []
            for inst in bb.instructions:
                si = inst.sync_info
                if si is not None and si.on_wait and len(si.on_wait) > _MAX_WAITS:
                    waits = list(si.on_wait)
                    keep = waits[-_MAX_WAITS:]
                    spill = waits[:-_MAX_WAITS]
                    for j in range(0, len(spill), _MAX_WAITS):
                        chunk = spill[j:j + _MAX_WAITS]
                        nop = mybir.InstNoOp(
                            name=f"{inst.name}-wsp{j}",
                            engine=inst.engine,
                            ins=[], outs=[],
                            sync_info=mybir.SyncInfo(on_wait=chunk, on_update=[]),
                        )
                        nc.register_instruction(nop, overwrite=True)
                        new_insts.append(nop)
                    inst.sync_info = mybir.SyncInfo(
                        on_wait=keep, on_update=list(si.on_update or []))
                new_insts.append(inst)
            bb.instructions[:] = new_insts


def build_nc(act_diag_mod=8, gtbufs=5, dgbufs=24, ngsplit=2):
    nc = bacc.Bacc()
    xg_in = nc.dram_tensor("xg", [C, XG], BF16, kind="ExternalInput")
    xt_in = nc.dram_tensor("xt", [HW, C], BF16, kind="ExternalInput")
    offw_in = nc.dram_tensor("offw", [C, K2 * 18], BF16, kind="ExternalInput")
    wmain_in = nc.dram_tensor("wmain", [C, K2 * O], BF16, kind="ExternalInput")
    biaso_in = nc.dram_tensor("biaso", [128, 1], F32, kind="ExternalInput")
    ybase_in = nc.dram_tensor("ybase", [128, FDIM], F32, kind="ExternalInput")
    xbase_in = nc.dram_tensor("xbase", [128, FDIM], F32, kind="ExternalInput")
    idb_in = nc.dram_tensor("identb", [128, 128], BF16, kind="ExternalInput")
    out_dram = nc.dram_tensor("out", [O, HW], F32, kind="ExternalOutput")

    with TileContext(nc) as tc:
        with tc.tile_pool(name="cst", bufs=1) as cst, \
             tc.tile_pool(name="fld", bufs=1) as fld, \
             tc.tile_pool(name="gth", bufs=gtbufs) as gth, \
             tc.tile_pool(name="dgp", bufs=dgbufs) as dgp, \
             tc.tile_pool(name="stb", bufs=1) as stb, \
             tc.tile_pool(name="otb", bufs=2) as otb:

            nc.gpsimd.load_library(library_config.mlp)

            # Tiny SWDGE op up front: bass barriers POOL's first dynamic DMA
            # against ALL outstanding HWDGE lanes; firing it now (nothing in
            # flight) keeps that barrier off the gather critical path.
            warm = cst.tile([16, 16], BF16, name="warm")
            nc.gpsimd.dma_start(warm[:, :], xg_in[0:16, 0:16])

            # ---- constant / input loads ----
            offw_sb = cst.tile([C, K2 * 18], BF16, name="offw_sb")
            nc.sync.dma_start(offw_sb[:, :], offw_in[:, :])
            wmain_sb = cst.tile([C, K2 * O], BF16, name="wmain_sb")
            nc.sync.dma_start(wmain_sb[:, :], wmain_in[:, :])
            biaso_sb = cst.tile([128, 1], F32, name="biaso_sb")
            nc.sync.dma_start(biaso_sb[:, :], biaso_in[:, :])
            ybase_sb = cst.tile([128, FDIM], F32, name="ybase_sb")
            nc.sync.dma_start(ybase_sb[:, :], ybase_in[:, :])
            xbase_sb = cst.tile([128, FDIM], F32, name="xbase_sb")
            nc.sync.dma_start(xbase_sb[:, :], xbase_in[:, :])
            identb = cst.tile([128, 128], BF16, name="identb")
            nc.sync.dma_start(identb[:, :], idb_in[:, :])
            xg_sb = cst.tile([C, XG], BF16, name="xg_sb")
            nc.sync.dma_start(xg_sb[:, :], xg_in[:, :])

            # ---- offset conv, transposed: offT[p, c*18 + j] ----
            # lhsT = guarded x pixels for chunk c shifted by tap (ky,kx);
            # rhs = offw tap slice [C, 18]; out free dim = 18.
            offT = fld.tile([128, NCH * 18], F32, name="offT")
            xg3 = xg_sb[:, :].rearrange("c (r w) -> c r w", w=GP)
            pso_cm = tc.tile_pool(name="pso", bufs=2, space="PSUM")
            pso = pso_cm.__enter__()
            for cg in range(8):
                ps = pso.tile([128, 8 * 18], F32, name=f"offps{cg}", tag="offps")
                ps_r = ps[:, :].rearrange("p (c4 two j) -> p two c4 j", two=2, j=18)
                for c4 in range(4):
                    c = cg * 4 + c4
                    for r in range(2):
                        for k in range(K2):
                            ky, kx = k // 3, k % 3
                            lhs = xg3[:, 2 * c + r + ky, kx: kx + 64]
                            nc.tensor.matmul(
                                ps[r * 64:(r + 1) * 64,
                                   (c4 * 2 + r) * 18:(c4 * 2 + r + 1) * 18],
                                lhs,
                                offw_sb[:, k * 18:(k + 1) * 18],
                                start=(k == 0), stop=(k == K2 - 1))
                offT_r = offT[:, :].rearrange("p (c j) -> p c j", j=18)
                for r in range(2):
                    nc.vector.tensor_copy(
                        offT_r[r * 64:(r + 1) * 64, cg * 4:(cg + 1) * 4],
                        ps_r[r * 64:(r + 1) * 64, r])
            pso_cm.__exit__(None, None, None)

            # ---- bilinear fields (fp32, [128, (c,k)=288]) ----
            offT4 = offT[:, :].rearrange("p (c k two) -> p two c k", two=2, k=K2)
            yb3 = ybase_sb[:, :].rearrange("p (c k) -> p c k", k=K2)
            xb3 = xbase_sb[:, :].rearrange("p (c k) -> p c k", k=K2)

            def f3(name):
                t = fld.tile([128, FDIM], F32, name=name, tag=name)
                return t, t[:, :].rearrange("p (c k) -> p c k", k=K2)

            VA = mybir.AluOpType

            # ==== pass 1: index path only (gates the gathers) ====
            srg = {}
            for ax in ("y", "x"):
                s, s3 = f3(f"s_{ax}")
                base3 = yb3 if ax == "y" else xb3
                nc.vector.tensor_tensor(s3, offT4[:, 0 if ax == "y" else 1], base3, VA.add)
                r, r3 = f3(f"r_{ax}")
                nc.vector.tensor_scalar_add(r[:, :], s[:, :], MAGIC)
                nc.vector.tensor_scalar_add(r[:, :], r[:, :], -MAGIC)
                g, g3 = f3(f"g_{ax}")
                nc.vector.tensor_tensor(g[:, :], r[:, :], s[:, :], VA.is_gt)
                i0, _ = f3(f"i0_{ax}")
                nc.vector.tensor_tensor(i0[:, :], r[:, :], g[:, :], VA.subtract)
                srg[ax] = (s, i0)
            iy0, ix0f = srg["y"][1], srg["x"][1]
            iy1, _ = f3("i1_y")
            nc.vector.tensor_scalar_add(iy1[:, :], iy0[:, :], 1.0)
            cy = []
            for a, ii in ((0, iy0), (1, iy1)):
                cc, _ = f3(f"c_y_{a}")
                nc.vector.tensor_scalar(cc[:, :], ii[:, :], 0.0, float(H - 1),
                                        VA.max, VA.min)
                cy.append(cc)
            bx, _ = f3("bx")
            nc.vector.tensor_scalar(bx[:, :], ix0f[:, :], 0.0, float(W - 2),
                                    VA.max, VA.min)
            # pair row indices idx = cy*64 + bx
            cys = []
            for a in range(2):
                cs, _ = f3(f"cys{a}")
                nc.vector.tensor_scalar_mul(cs[:, :], cy[a][:, :], float(W))
                cys.append(cs)
            # fidx col = ((k*2+a)*NH + h)*CPH + j  (chunk c = h*CPH + j)
            fidx = fld.tile([128, 2 * FDIM], F32, name="fidx")
            fidx_r = fidx[:, :].rearrange("p (k a h j) -> p a h j k",
                                          k=K2, a=2, h=NH, j=CPH)
            for a in range(2):
                nc.vector.tensor_tensor(fidx_r[:, a],
                                        cys[a][:, :].rearrange(
                                            "p (h j k) -> p h j k",
                                            h=NH, j=CPH, k=K2),
                                        bx[:, :].rearrange(
                                            "p (h j k) -> p h j k",
                                            h=NH, j=CPH, k=K2), VA.add)
            fidxi = fld.tile([128, 2 * FDIM], I16, name="fidxi")
            nc.vector.tensor_copy(fidxi[:, :], fidx[:, :])

            # ---- fold indices into SWDGE wrapped layout (ACT copies) ----
            # idxw col = kahj*8 + f; value stream for (k,a,h): i = j*128 + p
            # -> wrapped (i%16 = p%16, i//16 = j*8 + p//16)
            # stage 1: collapse partitions 128->16 with contiguous DMAs
            # (f-major staging layout), ~51ns each
            NKAHJ = 2 * FDIM  # 576
            stg = fld.tile([128, 8 * NKAHJ], I16, name="idxstg")
            for f in range(8):
                nc.scalar.dma_start(stg[0:16, f * NKAHJ:(f + 1) * NKAHJ],
                                    fidxi[16 * f:16 * (f + 1), :])
            # stage 2: in-partition column transpose (f, kahj) -> (kahj, f)
            idxw = fld.tile([128, NKAHJ * 8], I16, name="idxw")
            stg_r = stg[:, :].rearrange("p (f kahj) -> p kahj f", f=8)
            idxw_r = idxw[:, :].rearrange("p (kahj f) -> p kahj f", f=8)
            nc.scalar.copy(idxw_r[0:16, :NKAHJ // 2], stg_r[0:16, :NKAHJ // 2])
            nc.vector.tensor_copy(idxw_r[0:16, NKAHJ // 2:],
                                  stg_r[0:16, NKAHJ // 2:])
            # replicate the 16 wrapped partitions across all 128 (ACT HWDGE)
            for f in range(1, 8):
                nc.scalar.dma_start(idxw[16 * f:16 * (f + 1), :], idxw[0:16, :])

            # ==== pass 2: weight path (overlaps the first gathers) ====
            axes_w = {}
            for ax in ("y", "x"):
                s, i0 = srg[ax]
                fr, _ = f3(f"fr_{ax}")
                nc.vector.tensor_tensor(fr[:, :], s[:, :], i0[:, :], VA.subtract)
                i1 = iy1 if ax == "y" else None
                if i1 is None:
                    i1, _ = f3("i1_x")
                    nc.vector.tensor_scalar_add(i1[:, :], i0[:, :], 1.0)
                w_m = []
                for (ii, frac_is_w) in ((i0, False), (i1, True)):
                    v, _ = f3(f"v_{ax}_{frac_is_w}")
                    nc.vector.tensor_scalar(v[:, :], ii[:, :], 0.0, None, VA.is_ge)
                    t2, _ = f3(f"t2_{ax}_{frac_is_w}")
                    nc.vector.tensor_scalar(t2[:, :], ii[:, :], float(H - 1), None, VA.is_le)
                    nc.vector.tensor_tensor(v[:, :], v[:, :], t2[:, :], VA.mult)
                    wm, _ = f3(f"wm_{ax}_{frac_is_w}")
                    if frac_is_w:
                        nc.vector.tensor_tensor(wm[:, :], fr[:, :], v[:, :], VA.mult)
                    else:
                        nc.vector.tensor_scalar(wm[:, :], fr[:, :], -1.0, 1.0,
                                                VA.mult, VA.add)
                        nc.vector.tensor_tensor(wm[:, :], wm[:, :], v[:, :], VA.mult)
                    w_m.append(wm)
                axes_w[ax] = w_m
            wy, wx = axes_w["y"], axes_w["x"]
            dif, _ = f3("dif")
            nc.vector.tensor_tensor(dif[:, :], bx[:, :], ix0f[:, :], VA.subtract)
            eqA, _ = f3("eqA")
            nc.vector.tensor_scalar(eqA[:, :], dif[:, :], 0.0, None, VA.is_equal)
            eqB, _ = f3("eqB")
            nc.vector.tensor_scalar(eqB[:, :], dif[:, :], 1.0, None, VA.is_equal)
            eqC, _ = f3("eqC")
            nc.vector.tensor_scalar(eqC[:, :], dif[:, :], -1.0, None, VA.is_equal)
            WL, _ = f3("WL")
            WR, _ = f3("WR")
            t1, _ = f3("t1")
            nc.vector.tensor_tensor(WL[:, :], wx[0][:, :], eqA[:, :], VA.mult)
            nc.vector.tensor_tensor(t1[:, :], wx[1][:, :], eqB[:, :], VA.mult)
            nc.vector.tensor_tensor(WL[:, :], WL[:, :], t1[:, :], VA.add)
            nc.vector.tensor_tensor(WR[:, :], wx[1][:, :], eqA[:, :], VA.mult)
            nc.vector.tensor_tensor(t1[:, :], wx[0][:, :], eqC[:, :], VA.mult)
            nc.vector.tensor_tensor(WR[:, :], WR[:, :], t1[:, :], VA.add)
            # weights per (a, side): wcor2[a*2+side]
            wcor2 = []
            for a in range(2):
                for sd, Wside in ((0, WL), (1, WR)):
                    wc, _ = f3(f"wc{a}{sd}")
                    nc.vector.tensor_tensor(wc[:, :], wy[a][:, :], Wside[:, :], VA.mult)
                    wcor2.append(wc)

            # ---- gather + diag-matmul accumulate + GEMM ----
            xsrc = xt_in[:, :]
            xpairs = bass.AP(tensor=xsrc.tensor, offset=xsrc.offset,
                             ap=[[C, HW - 1], [1, 2 * C]])
            psp_cm = tc.tile_pool(name="ps", bufs=2, space="PSUM")
            psp = psp_cm.__enter__()
            pso2_cm = tc.tile_pool(name="pso2", bufs=1, space="PSUM")
            pso2 = pso2_cm.__enter__()
            ndiag = 0
            for h in range(NH):
                st_sb = stb.tile([128, K2 * CPH * 128], BF16,
                                 name=f"st{h}", tag="st")
                for k in range(K2):
                    gts = []
                    for a in range(2):
                        gt = gth.tile([128, CPH, 2 * C], BF16,
                                      name=f"g{h}_{k}_{a}", tag="gath")
                        base = ((k * 2 + a) * NH + h) * CPH * 8
                        ni = CPH * 128 // ngsplit
                        for g2 in range(ngsplit):
                            cpg = CPH // ngsplit
                            nc.gpsimd.dma_gather(
                                gt[:, g2 * cpg:(g2 + 1) * cpg, :], xpairs,
                                idxw[:, base + g2 * cpg * 8:
                                     base + (g2 + 1) * cpg * 8],
                                ni, ni, 2 * C, elem_step=C)
                        gts.append(gt)
                    st_tiles = []
                    for q in range(2):
                        st_ps = psp.tile([128, 8 * 128], F32,
                                         name=f"sp{h}_{k}_{q}", tag="stps")
                        st_tiles.append(st_ps)
                        for j8 in range(8):
                            j = q * 8 + j8
                            c = h * CPH + j
                            for a in range(2):
                                for sd in range(2):
                                    wcol = wcor2[a * 2 + sd][:, c * K2 + k:
                                                             c * K2 + k + 1]
                                    dg = dgp.tile([128, 128], BF16,
                                                  name=f"d{h}_{k}_{q}_{a}_{sd}_{j8}",
                                                  tag="diag")
                                    if act_diag_mod and ndiag % act_diag_mod == 0:
                                        nc.scalar.activation(
                                            dg[:, :], identb[:, :],
                                            mybir.ActivationFunctionType.Copy,
                                            scale=wcol)
                                    else:
                                        nc.vector.tensor_scalar_mul(
                                            dg[:, :], identb[:, :], wcol)
                                    ndiag += 1
                                    nc.tensor.matmul(
                                        st_ps[:, j8 * 128:(j8 + 1) * 128],
                                        gts[a][:, j, sd * C:(sd + 1) * C],
                                        dg[:, :],
                                        start=(a == 0 and sd == 0),
                                        stop=(a == 1 and sd == 1))
                    for q in range(2):
                        nc.scalar.copy(
                            st_sb[:, (k * 2 + q) * 1024:(k * 2 + q + 1) * 1024],
                            st_tiles[q][:, :])
                # GEMM: out^T[o, p] = sum_k wmain_k^T @ S_k^T, + bias
                ot_ps = pso2.tile([128, CPH * 128], F32, name=f"ot{h}", tag="otps")
                for j in range(CPH):
                    q, j8 = j // 8, j % 8
                    for k in range(K2):
                        nc.tensor.matmul(
                            ot_ps[:, j * 128:(j + 1) * 128],
                            wmain_sb[:, k * O:(k + 1) * O],
                            st_sb[:, (k * 2 + q) * 1024 + j8 * 128:
                                  (k * 2 + q) * 1024 + (j8 + 1) * 128],
                            start=(k == 0), stop=(k == K2 - 1))
                ot_sb = otb.tile([128, CPH * 128], F32, name=f"ots{h}", tag="ots")
                nc.scalar.activation(ot_sb[:, :], ot_ps[:, :],
                                     mybir.ActivationFunctionType.Identity,
                                     bias=biaso_sb[:, 0:1])
                nc.sync.dma_start(out_dram[:, h * CPH * 128:(h + 1) * CPH * 128],
                                  ot_sb[:, :])
            pso2_cm.__exit__(None, None, None)
            psp_cm.__exit__(None, None, None)

    nc.compile()
    _split_excess_waits(nc)
    return nc


_NC_CACHE = None


def _get_nc():
    global _NC_CACHE
    if _NC_CACHE is None:
        _NC_CACHE = build_nc()
    return _NC_CACHE


def _host_inputs(x, offset_w, offset_b, weight, bias):
    bf = ml_dtypes.bfloat16
    offw = np.ascontiguousarray(
        offset_w.reshape(18, C, K2).transpose(1, 2, 0).reshape(C, K2 * 18)).astype(bf)
    wmain = np.ascontiguousarray(
        weight.reshape(O, C, K2).transpose(1, 2, 0).reshape(C, K2 * O)).astype(bf)
    biaso = bias.reshape(128, 1).astype(np.float32)
    pi = np.arange(128)
    cc = np.arange(NCH)
    kk = np.arange(K2)
    pix = cc[None, :, None] * 128 + pi[:, None, None]          # [128, 32, 1]
    ob = offset_b.reshape(K2, 2).astype(np.float32)
    ybase = (pix // W - 1 + (kk // 3)[None, None, :]
             + ob[None, None, :, 0]).reshape(128, FDIM).astype(np.float32)
    xbase = (pix % W - 1 + (kk % 3)[None, None, :]
             + ob[None, None, :, 1]).reshape(128, FDIM).astype(np.float32)
    identb = np.eye(128, dtype=bf)
    shared = dict(offw=offw, wmain=wmain, biaso=biaso,
                  ybase=ybase, xbase=xbase, identb=identb)
    maps = []
    for b in range(B):
        m = dict(shared)
        xb = x[b].reshape(C, H, W)
        P = np.zeros((C, H + 2, GP), np.float32)
        P[:, 1:H + 1, 1:W + 1] = xb
        m["xg"] = P.reshape(C, XG).astype(bf)
        m["xt"] = np.ascontiguousarray(x[b].reshape(C, HW).T).astype(bf)
        maps.append(m)
    return maps


def kernel(x, offset_w, offset_b, weight, bias):
    from concourse.bass_utils import run_bass_kernel_spmd
    nc = _get_nc()
    in_maps = _host_inputs(np.asarray(x, np.float32), np.asarray(offset_w, np.float32),
                           np.asarray(offset_b, np.float32),
                           np.asarray(weight, np.float32), np.asarray(bias, np.float32))
    res = run_bass_kernel_spmd(nc, in_maps, core_ids=list(range(B)))
    out = np.stack([np.asarray(res.results[b]["out"], np.float32).reshape(O, H, W)
                    for b in range(B)])
    return out
